# revision 14
# baseline (speedup 1.0000x reference)
"""Discounted cumsum (B,H,S,D)=(8,16,4096,128), gamma per head, scan along S.

Strategy: batch-parallel across 8 NeuronCores (1 batch each, all 16 heads).
Per head, a two-level chunked scan implemented with PE matmuls (f32r):
  - block size T=128 along S -> 32 blocks per head, processed 4-at-a-time
    (tiles of [128 part = row-in-block, 512 free = 4 blocks x 128 d]).
  - s_k = w^T X_k   (block discounted sums)       [8 matmuls, N=512]
  - c   = AB @ s    (block-level scan, 32x32)     [1 matmul]
  - Y_k = A @ X_k + gvec (x) c_k                  [8+8 matmuls, N=512]
All matmul operands are float32r (1 cyc/row at N>=512); accumulation fp32.
"""
import sys

sys.path.insert(0, "/opt/trn_rl_repo")
import numpy as np

B, H, S, D = 8, 16, 4096, 128
T = 128          # block length along S
KB = S // T      # 32 blocks per head
TILE = 4 * T     # 512 free columns = 4 blocks per matmul
NT = S // TILE   # 8 tiles per head
SKEW_C = 1       # head-pipeline skew for the carry stage
SKEW_B = 3       # head-pipeline skew for the output stage

_CACHE = {}


def _build(repeat=1, mode="full"):
    import contextlib

    import concourse.bacc as bacc
    import concourse.tile as tile
    from concourse import mybir

    f32 = mybir.dt.float32
    f32r = mybir.dt.float32r

    nc = bacc.Bacc("TRN2", target_bir_lowering=False, debug=False)

    x_in = nc.declare_dram_parameter("x", [H, S, D], f32r, isOutput=False)
    at_in = nc.declare_dram_parameter("at", [T, H * T], f32r, isOutput=False)
    w_in = nc.declare_dram_parameter("w", [T, H], f32r, isOutput=False)
    gv_in = nc.declare_dram_parameter("gv", [1, H * T], f32r, isOutput=False)
    abt_in = nc.declare_dram_parameter("abt", [KB, H * KB], f32r, isOutput=False)
    y_out = nc.declare_dram_parameter("y", [H, S, D], f32, isOutput=True)

    with tile.TileContext(nc) as tc:
        with (
            tc.tile_pool(name="const", bufs=1) as const_pool,
            tc.tile_pool(name="xp", bufs=5) as x_pool,
            tc.tile_pool(name="op", bufs=2) as out_pool,
            tc.tile_pool(name="small", bufs=3) as small_pool,
            tc.tile_pool(name="sstage", bufs=4) as sstage_pool,
            tc.tile_pool(name="cflp", bufs=2) as cfl_pool,
            tc.tile_pool(name="sps", bufs=2, space="PSUM") as s_psum,
            tc.tile_pool(name="cps", bufs=2, space="PSUM") as c_psum,
            tc.tile_pool(name="yps", bufs=4, space="PSUM") as y_psum,
        ):
            at_sb = const_pool.tile([T, H * T], f32r)
            w_sb = const_pool.tile([T, H], f32r)
            gv_sb = const_pool.tile([1, H * T], f32r)
            abt_sb = const_pool.tile([KB, H * KB], f32r)
            nc.sync.dma_start(out=at_sb[:], in_=at_in[:])
            nc.sync.dma_start(out=w_sb[:], in_=w_in[:])
            nc.sync.dma_start(out=gv_sb[:], in_=gv_in[:])
            nc.sync.dma_start(out=abt_sb[:], in_=abt_in[:])

            xt = [None] * H      # per-head X tiles [128, 4096], free = (block, d)
            yt = [None] * H      # per-head output staging [128, 4096]
            s32 = [None] * H     # S as [KB, D]
            c32 = [None] * H     # C as [KB, D]
            cfl = [None] * H     # C_flat [1, KB*D]

            def stage_in(h):
                xt[h] = x_pool.tile([T, S], f32r, name=f"xt{h}", tag="xt")
                src = x_in[h].rearrange("(hf k p) d -> hf p k d", k=KB // 2, p=T)
                for hf in range(2):
                    dst = xt[h][:, hf * 2048 : (hf + 1) * 2048].rearrange(
                        "p (k d) -> p k d", d=D
                    )
                    nc.sync.dma_start(out=dst, in_=src[hf])

            def stage_s(h):
                s32[h] = small_pool.tile([KB, D], f32r, name=f"s32{h}", tag="s32")
                for t in range(NT):
                    s_ps = s_psum.tile([1, TILE], mybir.dt.float32, name="sps", tag="sps")
                    nc.tensor.matmul(
                        s_ps[:],
                        w_sb[:, h : h + 1],
                        xt[h][:, t * TILE : (t + 1) * TILE],
                        start=True,
                        stop=True,
                    )
                    s_sb = sstage_pool.tile([1, TILE], f32r, name="ssb", tag="ssb")
                    nc.scalar.copy(out=s_sb[:], in_=s_ps[:])
                    nc.scalar.dma_start(out=s32[h][4 * t : 4 * t + 4, :], in_=s_sb[:])

            def stage_c(h):
                c_ps = c_psum.tile([KB, D], mybir.dt.float32, name="cps", tag="cps")
                nc.tensor.matmul(
                    c_ps[:],
                    abt_sb[:, h * KB : (h + 1) * KB],
                    s32[h][:],
                    start=True,
                    stop=True,
                )
                c32[h] = small_pool.tile([KB, D], f32r, name=f"c32{h}", tag="c32")
                nc.scalar.copy(out=c32[h][:], in_=c_ps[:])
                cfl[h] = cfl_pool.tile([1, KB * D], f32r, name=f"cf{h}", tag="cf")
                nc.scalar.dma_start(out=cfl[h][:], in_=c32[h][:])

            def stage_b(h):
                yt[h] = out_pool.tile([T, S], mybir.dt.float32, name=f"yt{h}", tag="yt")
                for t in range(NT):
                    y_ps = y_psum.tile(
                        [T, TILE], mybir.dt.float32, name="yps", tag="yps"
                    )
                    nc.tensor.matmul(
                        y_ps[:],
                        at_sb[:, h * T : (h + 1) * T],
                        xt[h][:, t * TILE : (t + 1) * TILE],
                        start=True,
                        stop=False,
                    )
                    nc.tensor.matmul(
                        y_ps[:],
                        gv_sb[0:1, h * T : (h + 1) * T],
                        cfl[h][0:1, t * TILE : (t + 1) * TILE],
                        start=False,
                        stop=True,
                    )
                    nc.vector.tensor_copy(
                        out=yt[h][:, t * TILE : (t + 1) * TILE], in_=y_ps[:]
                    )
                    if t % 2 == 1:
                        q = t // 2
                        dst = y_out[h].rearrange("(q k p) d -> q p k d", k=NT, p=T)[q]
                        src = yt[h][:, q * 1024 : (q + 1) * 1024].rearrange(
                            "p (k d) -> p k d", d=D
                        )
                        nc.scalar.dma_start(out=dst, in_=src)

            def stage_dma_out(h):
                # store xt straight back (bitcast f32r view of y): DMA floor probe
                for q in range(4):
                    dst = y_out[h].rearrange("(q k p) d -> q p k d", k=NT, p=T)[
                        q
                    ].bitcast(f32r)
                    src = xt[h][:, q * 1024 : (q + 1) * 1024].rearrange(
                        "p (k d) -> p k d", d=D
                    )
                    nc.scalar.dma_start(out=dst, in_=src)

            loop = tc.For_i(0, repeat, 1) if repeat > 1 else contextlib.nullcontext()
            with loop:
                if mode == "dmaonly":
                    for i in range(H):
                        stage_in(i)
                        stage_dma_out(i)
                else:
                    for i in range(H + SKEW_B):
                        if i < H:
                            stage_in(i)
                            stage_s(i)
                        if 0 <= i - SKEW_C < H:
                            stage_c(i - SKEW_C)
                        if 0 <= i - SKEW_B < H:
                            stage_b(i - SKEW_B)

    nc.compile()
    return nc


def _constants(gamma):
    g = gamma.astype(np.float64)  # [H]
    i = np.arange(T)
    # A_h[i, s] = g^(i-s) for i>=s ; AT[s, h*T+i] = A_h[i, s]
    diff = i[:, None] - i[None, :]  # [i, s]
    at = np.zeros((T, H * T), np.float64)
    w = np.zeros((T, H), np.float64)
    gv = np.zeros((1, H * T), np.float64)
    abt = np.zeros((KB, H * KB), np.float64)
    k = np.arange(KB)
    kdiff = k[None, :] - k[:, None] - 1  # [j, k] -> k-1-j
    for h in range(H):
        gh = g[h]
        a_h = np.where(diff >= 0, gh ** np.maximum(diff, 0), 0.0)  # [i, s]
        at[:, h * T : (h + 1) * T] = a_h.T
        w[:, h] = gh ** (T - 1 - i)
        gv[0, h * T : (h + 1) * T] = gh ** (i + 1)
        G = gh ** T
        abt[:, h * KB : (h + 1) * KB] = np.where(
            kdiff >= 0, G ** np.maximum(kdiff, 0), 0.0
        )
    return (
        at.astype(np.float32),
        w.astype(np.float32),
        gv.astype(np.float32),
        abt.astype(np.float32),
    )


def _run(tensor, gamma, trace=False, repeat=1):
    from concourse.bass_utils import run_bass_kernel_spmd

    key = f"nc{repeat}"
    if key not in _CACHE:
        _CACHE[key] = _build(repeat)
    nc = _CACHE[key]

    at, w, gv, abt = _constants(np.asarray(gamma))
    tensor = np.asarray(tensor, dtype=np.float32)
    in_maps = [
        {
            "x": np.ascontiguousarray(tensor[c]),
            "at": at,
            "w": w,
            "gv": gv,
            "abt": abt,
        }
        for c in range(B)
    ]
    res = run_bass_kernel_spmd(nc, in_maps, core_ids=list(range(B)), trace=trace)
    y = np.stack([res.results[c]["y"] for c in range(B)], axis=0)
    return y, res


def kernel(tensor, gamma):
    y, _ = _run(tensor, gamma)
    return y


# revision 25
# speedup vs baseline: 1.2531x; 1.2531x over previous
"""Discounted cumsum (B,H,S,D)=(8,16,4096,128), gamma per head, scan along S.

Strategy: batch-parallel across 8 NeuronCores (1 batch each, all 16 heads).
Per head, a two-level chunked scan implemented with PE matmuls (f32r):
  - block size T=128 along S -> 32 blocks per head, processed 4-at-a-time
    (tiles of [128 part = row-in-block, 512 free = 4 blocks x 128 d]).
  - s_k = w^T X_k   (block discounted sums)       [8 matmuls, N=512]
  - c   = AB @ s    (block-level scan, 32x32)     [1 matmul]
  - Y_k = A @ X_k + gvec (x) c_k                  [8+8 matmuls, N=512]
All matmul operands are float32r (1 cyc/row at N>=512); accumulation fp32.
"""
import sys

sys.path.insert(0, "/opt/trn_rl_repo")
import numpy as np

B, H, S, D = 8, 16, 4096, 128
T = 128          # block length along S
KB = S // T      # 32 blocks per head
TILE = 4 * T     # 512 free columns = 4 blocks per matmul
NT = S // TILE   # 8 tiles per head
SKEW_C = 1       # head-pipeline skew for the carry stage
SKEW_B = 3       # head-pipeline skew for the output stage

_CACHE = {}


def _build(repeat=1, mode="full"):
    import contextlib

    import concourse.bacc as bacc
    import concourse.tile as tile
    from concourse import mybir

    f32 = mybir.dt.float32
    f32r = mybir.dt.float32r

    nc = bacc.Bacc("TRN2", target_bir_lowering=False, debug=False)

    x_in = nc.declare_dram_parameter("x", [H, S, D], f32r, isOutput=False)
    at_in = nc.declare_dram_parameter("at", [T, H * T], f32r, isOutput=False)
    w_in = nc.declare_dram_parameter("w", [T, H], f32r, isOutput=False)
    gv_in = nc.declare_dram_parameter("gv", [1, H * T], f32r, isOutput=False)
    abt_in = nc.declare_dram_parameter("abt", [KB, H * KB], f32r, isOutput=False)
    y_out = nc.declare_dram_parameter("y", [H, S, D], f32, isOutput=True)

    with tile.TileContext(nc) as tc:
        with (
            tc.tile_pool(name="const", bufs=1) as const_pool,
            tc.tile_pool(name="xp", bufs=5) as x_pool,
            tc.tile_pool(name="op", bufs=2) as out_pool,
            tc.tile_pool(name="small", bufs=3) as small_pool,
            tc.tile_pool(name="sstage", bufs=2) as sstage_pool,
            tc.tile_pool(name="cflp", bufs=2) as cfl_pool,
            tc.tile_pool(name="sps", bufs=3, space="PSUM") as s_psum,
            tc.tile_pool(name="cps", bufs=1, space="PSUM") as c_psum,
            tc.tile_pool(name="yps", bufs=4, space="PSUM") as y_psum,
        ):
            at_sb = const_pool.tile([T, H * T], f32r)
            w_sb = const_pool.tile([T, H], f32r)
            gv_sb = const_pool.tile([1, H * T], f32r)
            abt_sb = const_pool.tile([KB, H * KB], f32r)
            nc.sync.dma_start(out=at_sb[:], in_=at_in[:])
            nc.sync.dma_start(out=w_sb[:], in_=w_in[:])
            nc.sync.dma_start(out=gv_sb[:], in_=gv_in[:])
            nc.sync.dma_start(out=abt_sb[:], in_=abt_in[:])

            xt = [None] * H      # per-head X tiles [128, 4096], free = (block, d)
            yt = [None] * H      # per-head output staging [128, 4096]
            s32 = [None] * H     # S as [KB, D]
            c32 = [None] * H     # C as [KB, D]
            cfl = [None] * H     # C_flat [1, KB*D]

            def stage_in(h):
                xt[h] = x_pool.tile([T, S], f32r, name=f"xt{h}", tag="xt")
                src = x_in[h].rearrange("(hf k p) d -> hf p k d", k=KB // 2, p=T)
                for hf in range(2):
                    dst = xt[h][:, hf * 2048 : (hf + 1) * 2048].rearrange(
                        "p (k d) -> p k d", d=D
                    )
                    nc.sync.dma_start(out=dst, in_=src[hf])

            def stage_s(h):
                s32[h] = small_pool.tile([KB, D], f32r, name=f"s32{h}", tag="s32")
                s_fl = sstage_pool.tile([1, KB * D], f32r, name="sfl", tag="sfl")
                for t in range(NT):
                    s_ps = s_psum.tile([1, TILE], mybir.dt.float32, name="sps", tag="sps")
                    nc.tensor.matmul(
                        s_ps[:],
                        w_sb[:, h : h + 1],
                        xt[h][:, t * TILE : (t + 1) * TILE],
                        start=True,
                        stop=True,
                    )
                    nc.scalar.copy(
                        out=s_fl[0:1, t * TILE : (t + 1) * TILE], in_=s_ps[:]
                    )
                nc.gpsimd.dma_start(out=s32[h][:], in_=s_fl[:])

            def stage_c(h):
                c_ps = c_psum.tile([KB, D], mybir.dt.float32, name="cps", tag="cps")
                nc.tensor.matmul(
                    c_ps[:],
                    abt_sb[:, h * KB : (h + 1) * KB],
                    s32[h][:],
                    start=True,
                    stop=True,
                )
                c32[h] = small_pool.tile([KB, D], f32r, name=f"c32{h}", tag="c32")
                nc.scalar.copy(out=c32[h][:], in_=c_ps[:])
                cfl[h] = cfl_pool.tile([1, KB * D], f32r, name=f"cf{h}", tag="cf")
                nc.gpsimd.dma_start(out=cfl[h][:], in_=c32[h][:])

            def stage_b(h):
                yt[h] = out_pool.tile([T, S], mybir.dt.float32, name=f"yt{h}", tag="yt")
                for t in range(NT):
                    y_ps = y_psum.tile(
                        [T, TILE], mybir.dt.float32, name="yps", tag="yps"
                    )
                    nc.tensor.matmul(
                        y_ps[:],
                        at_sb[:, h * T : (h + 1) * T],
                        xt[h][:, t * TILE : (t + 1) * TILE],
                        start=True,
                        stop=(mode == "nocarry"),
                    )
                    if mode != "nocarry":
                        nc.tensor.matmul(
                            y_ps[:],
                            gv_sb[0:1, h * T : (h + 1) * T],
                            cfl[h][0:1, t * TILE : (t + 1) * TILE],
                            start=False,
                            stop=True,
                        )
                    nc.vector.tensor_copy(
                        out=yt[h][:, t * TILE : (t + 1) * TILE], in_=y_ps[:]
                    )
                    if t % 2 == 1 and mode != "computeonly":
                        q = t // 2
                        dst = y_out[h].rearrange("(q k p) d -> q p k d", k=NT, p=T)[q]
                        src = yt[h][:, q * 1024 : (q + 1) * 1024].rearrange(
                            "p (k d) -> p k d", d=D
                        )
                        nc.scalar.dma_start(out=dst, in_=src)

            def stage_dma_out(h):
                # store xt straight back (bitcast f32r view of y): DMA floor probe
                for q in range(4):
                    dst = y_out[h].rearrange("(q k p) d -> q p k d", k=NT, p=T)[
                        q
                    ].bitcast(f32r)
                    src = xt[h][:, q * 1024 : (q + 1) * 1024].rearrange(
                        "p (k d) -> p k d", d=D
                    )
                    nc.scalar.dma_start(out=dst, in_=src)

            if mode == "computeonly":
                xconst = const_pool.tile([T, S], f32r)
                nc.vector.memset(xconst[:].bitcast(f32), 0.125)

                def stage_in(h):  # noqa: F811
                    xt[h] = xconst

                def _no_store(h, q):
                    return

            loop = tc.For_i(0, repeat, 1) if repeat > 1 else contextlib.nullcontext()
            with loop:
                if mode == "dmaonly":
                    for i in range(H):
                        stage_in(i)
                        stage_dma_out(i)
                elif mode == "nocarry":
                    for i in range(H + 1):
                        if i < H:
                            stage_in(i)
                        if 0 <= i - 1 < H:
                            stage_b(i - 1)
                else:
                    for i in range(H + SKEW_B):
                        if i < H:
                            stage_in(i)
                            stage_s(i)
                        if 0 <= i - SKEW_C < H:
                            stage_c(i - SKEW_C)
                        if 0 <= i - SKEW_B < H:
                            stage_b(i - SKEW_B)

    nc.compile()
    return nc


def _constants(gamma):
    g = gamma.astype(np.float64)  # [H]
    i = np.arange(T)
    # A_h[i, s] = g^(i-s) for i>=s ; AT[s, h*T+i] = A_h[i, s]
    diff = i[:, None] - i[None, :]  # [i, s]
    at = np.zeros((T, H * T), np.float64)
    w = np.zeros((T, H), np.float64)
    gv = np.zeros((1, H * T), np.float64)
    abt = np.zeros((KB, H * KB), np.float64)
    k = np.arange(KB)
    kdiff = k[None, :] - k[:, None] - 1  # [j, k] -> k-1-j
    for h in range(H):
        gh = g[h]
        a_h = np.where(diff >= 0, gh ** np.maximum(diff, 0), 0.0)  # [i, s]
        at[:, h * T : (h + 1) * T] = a_h.T
        w[:, h] = gh ** (T - 1 - i)
        gv[0, h * T : (h + 1) * T] = gh ** (i + 1)
        G = gh ** T
        abt[:, h * KB : (h + 1) * KB] = np.where(
            kdiff >= 0, G ** np.maximum(kdiff, 0), 0.0
        )
    return (
        at.astype(np.float32),
        w.astype(np.float32),
        gv.astype(np.float32),
        abt.astype(np.float32),
    )


def _run(tensor, gamma, trace=False, repeat=1):
    from concourse.bass_utils import run_bass_kernel_spmd

    key = f"nc{repeat}"
    if key not in _CACHE:
        _CACHE[key] = _build(repeat)
    nc = _CACHE[key]

    at, w, gv, abt = _constants(np.asarray(gamma))
    tensor = np.asarray(tensor, dtype=np.float32)
    in_maps = [
        {
            "x": np.ascontiguousarray(tensor[c]),
            "at": at,
            "w": w,
            "gv": gv,
            "abt": abt,
        }
        for c in range(B)
    ]
    res = run_bass_kernel_spmd(nc, in_maps, core_ids=list(range(B)), trace=trace)
    y = np.stack([res.results[c]["y"] for c in range(B)], axis=0)
    return y, res


def kernel(tensor, gamma):
    y, _ = _run(tensor, gamma)
    return y
